# revision 19
# baseline (speedup 1.0000x reference)
"""Multi-head attention (B=2, S=2048, D=1024, H=16) on 8 TRN2 NeuronCores.

Sharding: batch x head-group. Core c handles batch c//4 and heads
[4*(c%4), 4*(c%4)+4). Each core computes its heads' Q/K/V projections
(column-parallel), causal attention, and a row-parallel partial of the
output projection. The host sums the 4 partials per batch and adds dense_b.

All matmul operands are fp16 (fp32 PSUM accumulation). On-core dataflow:
  QT/KT [128, 2, S] fp16: partition = head-pair-local feature (2 heads x 64),
    second dim = head pair (pc). V [128p=seq, chunk, head, 65] fp16 with a
    ones column (col 64) for the softmax denominator.
  per q-block j (512 wide), head pair pc, k-chunk kc (128 wide):
    L[:, i, off:] = KT_chunk.T @ QT_block   (2 heads row-packed in the PE,
      columns below the causal diagonal skipped)
    PT = exp(0.125 * L)  fp16  (ScalarE only does exp; diagonal 128-block
      masked multiplicatively with a 0/1 lower-tri pattern on DVE)
    per head i, per q-sub qs (128 wide, qs >= chunk diagonal): natural-
      orientation PV: O[i][:, qs, 0:65] += PT[:, i, qs-cols].T @ V_aug
      (full 128x128 PE payload, 65 streamed rows)
  per head: recip = 1/O[:, :, 64] (DVE); O8 = O * recip (fused PSUM evac);
    OT = PE-transpose(O8) per q-sub into a shared PSUM tile; one DVE copy
    to OT_sb [128, 2, 4, 128] fp16 (partition = dense contraction dim).
  dense: outT[mc*128:+128, q-block] = sum_t dnT[:, t, mc].T @ OT_sb[:, t]
    evacuated on the Pool engine (fp16) and DMA'd out.
"""

import numpy as np
from contextlib import ExitStack

import concourse.tile as tile
from concourse import bacc, mybir
from concourse.bass_utils import run_bass_kernel_spmd

F32 = mybir.dt.float32
F16 = mybir.dt.float16
AF = mybir.ActivationFunctionType
ADD = mybir.AluOpType.add
MULT = mybir.AluOpType.mult

B, S, D, H = 2, 2048, 1024, 16
NCORES = 8
HL = 4            # heads per core
DH = D // H       # 64
DLOC = HL * DH    # 256 local feature dims
SBK = 512         # seq block (q)
NSB = S // SBK    # 4
KCH = 128         # k chunk
NCH = S // KCH    # 16


def _ts(i, n):
    return slice(i * n, (i + 1) * n)


def build(debug=False):
    nc = bacc.Bacc(None, target_bir_lowering=False)

    xqT = nc.dram_tensor("xqT", [D, S], F16, kind="ExternalInput")
    xkT = nc.dram_tensor("xkT", [D, S], F16, kind="ExternalInput")
    xvT = nc.dram_tensor("xvT", [D, S], F16, kind="ExternalInput")
    wqT = nc.dram_tensor("wqT", [D, DLOC], F16, kind="ExternalInput")
    wkT = nc.dram_tensor("wkT", [D, DLOC], F16, kind="ExternalInput")
    wvT = nc.dram_tensor("wvT", [D, DLOC], F16, kind="ExternalInput")
    dnT = nc.dram_tensor("dnT", [DLOC, D], F16, kind="ExternalInput")
    outT = nc.dram_tensor("outT", [D, S], F16, kind="ExternalOutput")
    if debug:
        dQT = nc.dram_tensor("dQT", [128, 2, S], F16, kind="ExternalOutput")
        dKT = nc.dram_tensor("dKT", [128, 2, S], F16, kind="ExternalOutput")
        dV = nc.dram_tensor("dV", [128, NCH, HL, DH + 1], F16, kind="ExternalOutput")
        dPT = nc.dram_tensor("dPT", [128, 2, SBK], F16, kind="ExternalOutput")
        dO = nc.dram_tensor("dO", [2, 128, NSB, 128], F32, kind="ExternalOutput")
        dO8 = nc.dram_tensor("dO8", [2, 128, NSB, DH], F16, kind="ExternalOutput")
        dOT = nc.dram_tensor("dOT", [128, 2, NSB, 128], F16, kind="ExternalOutput")

    # lower-tri 0/1 pattern (allowed = k <= q within the diagonal block)
    tri_np = (np.arange(128)[:, None] <= np.arange(128)[None, :]).astype(np.float16)
    tri_c = nc.inline_tensor(tri_np, name="tri01")
    id_c = nc.inline_tensor(np.eye(128, dtype=np.float16), name="id128")

    with tile.TileContext(nc) as tc, ExitStack() as ctx:
        pers = ctx.enter_context(tc.tile_pool(name="pers", bufs=1))
        xpool = ctx.enter_context(tc.tile_pool(name="xpool", bufs=24))
        ptp = ctx.enter_context(tc.tile_pool(name="ptp", bufs=1))  # 16 tags x 1 buf
        o8p = ctx.enter_context(tc.tile_pool(name="o8p", bufs=4))
        otp = ctx.enter_context(tc.tile_pool(name="otp", bufs=2))
        evp = ctx.enter_context(tc.tile_pool(name="evp", bufs=5))
        smallp = ctx.enter_context(tc.tile_pool(name="smallp", bufs=4))
        mmp = ctx.enter_context(tc.tile_pool(name="mmp", bufs=2, space="PSUM"))
        lp = ctx.enter_context(tc.tile_pool(name="lp", bufs=2, space="PSUM"))
        onp = ctx.enter_context(tc.tile_pool(name="onp", bufs=1, space="PSUM"))  # 2 tags x 1 buf

        # ---------- persistent tiles ----------
        wparts = {}
        for wname in ("q", "k", "v"):
            wparts[wname] = [
                pers.tile([128, 2, DLOC], F16, tag=f"w{wname}{kc2}",
                          name=f"w_{wname}_{kc2}")
                for kc2 in range(4)
            ]
        dn_sb = pers.tile([128, 2, D], F16, tag="dn")
        tri_sb = pers.tile([128, 128], F16, tag="tri")
        id_sb = pers.tile([128, 128], F16, tag="id")
        nc.sync.dma_start(out=tri_sb, in_=tri_c[:, :])
        nc.sync.dma_start(out=id_sb, in_=id_c[:, :])

        QT_sb = pers.tile([128, 2, S], F16, tag="QT")
        KT_sb = pers.tile([128, 2, S], F16, tag="KT")
        V_sb = pers.tile([128, NCH, HL, DH + 1], F16, tag="V")
        # ones column (softmax denominator accumulates via PV matmul)
        nc.vector.memset(V_sb[:, :, :, DH:DH + 1], 1.0)

        outT_r = outT.rearrange("(c p) s -> p c s", p=128)

        def load_x(j, js):
            xt = {}
            for xname, src in (("q", xqT), ("k", xkT), ("v", xvT)):
                srcr = src.rearrange("(c p) s -> p c s", p=128)
                tiles = []
                for kc2 in range(4):
                    t = xpool.tile([128, 2, SBK], F16, tag="xt",
                                   name=f"x_{xname}_{j}_{kc2}")
                    nc.sync.dma_start(out=t, in_=srcr[:, _ts(kc2, 2), js])
                    tiles.append(t)
                xt[xname] = tiles
            return xt

        def phase_A(j, js, xt=None):
            # ---------- projections for s-block j ----------
            if xt is None:
                xt = load_x(j, js)

            # evacs on the Activation engine: it is idle during phase A
            for bname, dst in (("q", QT_sb), ("k", KT_sb)):
                for mc in range(2):
                    ps = mmp.tile([128, 512], F32, tag="mm")
                    for kc in range(8):
                        nc.tensor.matmul(
                            ps[:, :],
                            lhsT=wparts[bname][kc // 2][:, kc % 2, _ts(mc, 128)],
                            rhs=xt[bname][kc // 2][:, kc % 2, :],
                            start=(kc == 0), stop=(kc == 7),
                        )
                    nc.scalar.copy(dst[:, mc, js], ps)

            for sc in range(4):
                ps = mmp.tile([128, 512], F32, tag="mm")
                for kc in range(8):
                    nc.tensor.matmul(
                        ps[:, 0:DLOC],
                        lhsT=xt["v"][kc // 2][:, kc % 2, _ts(sc, 128)],
                        rhs=wparts["v"][kc // 2][:, kc % 2, :],
                        start=(kc == 0), stop=(kc == 7),
                    )
                nc.scalar.copy(
                    V_sb[:, j * 4 + sc, :, 0:DH],
                    ps[:, 0:DLOC].rearrange("p (h d) -> p h d", h=HL),
                )

        def phase_B(j, js):
            # ---------- attention + dense for q-block j ----------
            # Emission order is pipelined so the in-order PE queue always has
            # work while DVE runs the recip/normalize/evac chains, and so the
            # Act engine (exp-bound) is fed logits as early as possible.
            nkc = (j + 1) * 4
            OT = otp.tile([128, 2, NSB, 128], F16, tag="ot", name=f"OT_{j}")

            def emit_logits_exp(pc):
                PTs = []
                for kc in range(nkc):
                    off = max(0, kc - 4 * j) * KCH  # causal column trim
                    L = lp.tile([128, 2, SBK], F32, tag="L")
                    for i in range(2):
                        nc.tensor.matmul(
                            L[:, i, off:SBK],
                            lhsT=KT_sb[_ts(i, 64), pc, _ts(kc, KCH)],
                            rhs=QT_sb[_ts(i, 64), pc, j * SBK + off:(j + 1) * SBK],
                            start=True, stop=True,
                            tile_position=(64 * i, 0),
                        )
                    PT = ptp.tile([128, 2, SBK], F16, tag=f"PT{pc}_{kc}",
                                  name=f"PT_{j}_{pc}_{kc}")
                    nc.scalar.activation(
                        out=PT[:, :, off:SBK], in_=L[:, :, off:SBK],
                        func=AF.Exp, scale=0.125)
                    if kc >= 4 * j:
                        # mask the diagonal 128-block (0/1 lower-tri multiply)
                        nc.vector.tensor_tensor(
                            out=PT[:, :, off:off + KCH],
                            in0=PT[:, :, off:off + KCH],
                            in1=tri_sb[:, None, :].broadcast_to([128, 2, KCH]),
                            op=MULT,
                        )
                    if debug and j == 0 and pc == 0 and kc == 0:
                        nc.sync.dma_start(out=dPT[:, :, :], in_=PT)
                    PTs.append(PT)
                return PTs

            PTs = [emit_logits_exp(0), emit_logits_exp(1)]
            Os, O8s = {}, {}

            def emit_pv(pc, i):
                O = onp.tile([128, NSB, 128], F32, tag=f"o{i}",
                             name=f"O_{j}_{pc}_{i}")
                for qs in range(NSB):
                    kmax = 4 * j + qs
                    for kc in range(kmax + 1):
                        nc.tensor.matmul(
                            O[:, qs, 0:DH + 1],
                            lhsT=PTs[pc][kc][:, i, _ts(qs, 128)],
                            rhs=V_sb[:, kc, 2 * pc + i, :],
                            start=(kc == 0), stop=(kc == kmax),
                            skip_group_check=True,
                        )
                Os[pc, i] = O

            def emit_norm(pc, i):
                O = Os[pc, i]
                rc = smallp.tile([128, NSB, 1], F32, tag="rc")
                nc.vector.reciprocal(rc, O[:, :, DH:DH + 1])
                O8 = o8p.tile([128, NSB, DH], F16, tag=f"o8{i}",
                              name=f"O8_{j}_{pc}_{i}")
                nc.vector.tensor_tensor(
                    out=O8, in0=O[:, :, 0:DH],
                    in1=rc.broadcast_to([128, NSB, DH]), op=MULT,
                )
                if debug and j == 0 and pc == 0:
                    ostage = evp.tile([128, NSB, 128], F32, tag="ev",
                                      name=f"ostage_{i}")
                    nc.vector.tensor_copy(ostage, O)
                    nc.sync.dma_start(out=dO[i, :, :, :], in_=ostage)
                    nc.sync.dma_start(out=dO8[i, :, :, :], in_=O8)
                O8s[pc, i] = O8

            def emit_transposes(pc):
                tp = mmp.tile([128, NSB, 128], F16, tag="mm", name=f"tp_{j}_{pc}")
                for i in range(2):
                    for qs in range(NSB):
                        nc.tensor.transpose(
                            tp[_ts(i, 64), qs, :], O8s[pc, i][:, qs, :], id_sb,
                            tile_position=(0, 64 * i),
                        )
                nc.vector.tensor_copy(OT[:, pc, :, :], tp)

            emit_pv(0, 0)
            emit_norm(0, 0)
            emit_pv(0, 1)
            emit_norm(0, 1)
            emit_pv(1, 0)
            emit_norm(1, 0)
            emit_transposes(0)
            emit_pv(1, 1)
            emit_norm(1, 1)
            emit_transposes(1)
            if debug and j == 0:
                nc.sync.dma_start(out=dOT[:, :, :, :], in_=OT)

            for mc in range(8):
                dps = mmp.tile([128, 512], F32, tag="mm")
                for t in range(2):
                    nc.tensor.matmul(
                        dps[:, :],
                        lhsT=dn_sb[:, t, _ts(mc, 128)],
                        rhs=OT[:, t, :, :].rearrange("p a b -> p (a b)"),
                        start=(t == 0), stop=(t == 1),
                    )
                ev = evp.tile([128, 512], F16, tag="ev")
                # alternate evac engine: halves the evac-bound dense stretch
                if mc % 2 == 0:
                    nc.vector.tensor_copy(ev, dps)
                else:
                    nc.scalar.copy(ev, dps)
                nc.sync.dma_start(out=outT_r[:, mc, js], in_=ev)

        # startup: interleave weight-part and first-block x DMAs in
        # consumption order so the first projection matmuls start early
        xt0 = {}
        js0 = _ts(0, SBK)
        for xname, xsrc in (("q", xqT), ("k", xkT), ("v", xvT)):
            srcr = xsrc.rearrange("(c p) s -> p c s", p=128)
            wr = {"q": wqT, "k": wkT, "v": wvT}[xname].rearrange(
                "(c p) m -> p c m", p=128)
            tiles = []
            for kc2 in range(4):
                nc.sync.dma_start(out=wparts[xname][kc2], in_=wr[:, _ts(kc2, 2), :])
                t = xpool.tile([128, 2, SBK], F16, tag="xt",
                               name=f"x_{xname}_0_{kc2}")
                nc.sync.dma_start(out=t, in_=srcr[:, _ts(kc2, 2), js0])
                tiles.append(t)
            xt0[xname] = tiles
        nc.sync.dma_start(
            out=dn_sb, in_=dnT.rearrange("(t p) n -> p t n", p=128))

        # interleave A and B: B(j) only needs A(0..j), and phase B is
        # Act-(exp-)bound while phase A is PE-bound — interleaving keeps
        # both engines fed throughout
        xts = {0: xt0}
        for j in range(NSB):
            if j + 1 < NSB:
                xts[j + 1] = load_x(j + 1, _ts(j + 1, SBK))  # prefetch
            phase_A(j, _ts(j, SBK), xt=xts.pop(j))
            if debug and j == NSB - 1:
                nc.sync.dma_start(out=dQT[:, :, :], in_=QT_sb)
                nc.sync.dma_start(out=dKT[:, :, :], in_=KT_sb)
                nc.sync.dma_start(out=dV[:, :, :, :], in_=V_sb)
            phase_B(j, _ts(j, SBK))

    nc.finalize()
    return nc


_CACHE = {}


def _get_nc(causal=True, with_bq=False, with_bk=False, with_bv=False):
    key = (causal, with_bq, with_bk, with_bv)
    if key not in _CACHE:
        assert causal and not (with_bq or with_bk or with_bv)
        _CACHE[key] = build()
    return _CACHE[key]


def _numpy_fallback(query, key_, value, mask, wq_w, wq_b, wk_w, wk_b, wv_w,
                    wv_b, dense_w, dense_b):
    out = np.empty((B, S, D), np.float32)
    m4 = np.asarray(mask, np.float32).reshape(-1, S, S)
    for b in range(B):
        q = (query[b] @ wq_w.T + wq_b).reshape(S, H, DH).transpose(1, 0, 2)
        k = (key_[b] @ wk_w.T + wk_b).reshape(S, H, DH).transpose(1, 0, 2)
        v = (value[b] @ wv_w.T + wv_b).reshape(S, H, DH).transpose(1, 0, 2)
        mb = m4[min(b, m4.shape[0] - 1)]
        o = np.empty((H, S, DH), np.float32)
        for h in range(H):
            lg = (q[h] @ k[h].T) / np.sqrt(np.float32(DH)) + mb * np.float32(-1e9)
            lg -= lg.max(-1, keepdims=True)
            p = np.exp(lg)
            p /= p.sum(-1, keepdims=True)
            o[h] = p @ v[h]
        out[b] = o.transpose(1, 0, 2).reshape(S, D) @ dense_w.T + dense_b
    return out


def _prep_in_maps(query, key_, value, wq_w, wk_w, wv_w, dense_w):
    xT = {}
    for b in range(B):
        xT[b] = (
            np.ascontiguousarray(query[b].T).astype(np.float16),
            np.ascontiguousarray(key_[b].T).astype(np.float16),
            np.ascontiguousarray(value[b].T).astype(np.float16),
        )
    in_maps = []
    for c in range(NCORES):
        b, g = divmod(c, 4)
        sl = _ts(g, DLOC)
        in_maps.append({
            "xqT": xT[b][0], "xkT": xT[b][1], "xvT": xT[b][2],
            "wqT": np.ascontiguousarray(wq_w[sl].T).astype(np.float16),
            "wkT": np.ascontiguousarray(wk_w[sl].T).astype(np.float16),
            "wvT": np.ascontiguousarray(wv_w[sl].T).astype(np.float16),
            "dnT": np.ascontiguousarray(dense_w[:, sl].T).astype(np.float16),
        })
    return in_maps


def kernel(query, key_, value, mask, wq_w, wq_b, wk_w, wk_b, wv_w, wv_b,
           dense_w, dense_b, _profile_kw=None):
    query = np.asarray(query, np.float32)
    key_ = np.asarray(key_, np.float32)
    value = np.asarray(value, np.float32)
    mask2d = np.asarray(mask, np.float32).reshape(S, S)
    wq_w = np.asarray(wq_w, np.float32)
    wk_w = np.asarray(wk_w, np.float32)
    wv_w = np.asarray(wv_w, np.float32)
    dense_w = np.asarray(dense_w, np.float32)
    dense_b = np.asarray(dense_b, np.float32)

    causal = bool(np.array_equal(mask2d, np.triu(np.ones((S, S), np.float32), k=1)))
    if not causal or np.any(wq_b) or np.any(wk_b) or np.any(wv_b):
        out = _numpy_fallback(query, key_, value, mask, wq_w, wq_b, wk_w,
                              wk_b, wv_w, wv_b, dense_w, dense_b)
        return (out, None) if _profile_kw else out

    in_maps = _prep_in_maps(query, key_, value, wq_w, wk_w, wv_w, dense_w)
    nc = _get_nc(True, False, False, False)
    res = run_bass_kernel_spmd(nc, in_maps, core_ids=list(range(NCORES)),
                               **(_profile_kw or {}))

    out = np.empty((B, S, D), np.float32)
    for b in range(B):
        acc = res.results[4 * b]["outT"].astype(np.float32)
        for g in range(1, 4):
            acc = acc + res.results[4 * b + g]["outT"].astype(np.float32)
        out[b] = acc.T + dense_b[None, :]
    if _profile_kw:
        return out, res
    return out


# revision 20
# speedup vs baseline: 1.1250x; 1.1250x over previous
"""Multi-head attention (B=2, S=2048, D=1024, H=16) on 8 TRN2 NeuronCores.

Sharding: batch x head-group. Core c handles batch c//4 and heads
[4*(c%4), 4*(c%4)+4). Each core computes its heads' Q/K/V projections
(column-parallel), causal attention, and a row-parallel partial of the
output projection. The host sums the 4 partials per batch and adds dense_b.

All matmul operands are fp16 (fp32 PSUM accumulation). On-core dataflow:
  QT/KT [128, 2, S] fp16: partition = head-pair-local feature (2 heads x 64),
    second dim = head pair (pc). V [128p=seq, chunk, head, 65] fp16 with a
    ones column (col 64) for the softmax denominator.
  per q-block j (512 wide), head pair pc, k-chunk kc (128 wide):
    L[:, i, off:] = KT_chunk.T @ QT_block   (2 heads row-packed in the PE,
      columns below the causal diagonal skipped)
    PT = exp(0.125 * L)  fp16  (ScalarE only does exp; diagonal 128-block
      masked multiplicatively with a 0/1 lower-tri pattern on DVE)
    per head i, per q-sub qs (128 wide, qs >= chunk diagonal): natural-
      orientation PV: O[i][:, qs, 0:65] += PT[:, i, qs-cols].T @ V_aug
      (full 128x128 PE payload, 65 streamed rows)
  per head: recip = 1/O[:, :, 64] (DVE); O8 = O * recip (fused PSUM evac);
    OT = PE-transpose(O8) per q-sub into a shared PSUM tile; one DVE copy
    to OT_sb [128, 2, 4, 128] fp16 (partition = dense contraction dim).
  dense: outT[mc*128:+128, q-block] = sum_t dnT[:, t, mc].T @ OT_sb[:, t]
    evacuated on the Pool engine (fp16) and DMA'd out.
"""

import numpy as np
from contextlib import ExitStack

import concourse.tile as tile
from concourse import bacc, mybir
from concourse.bass_utils import run_bass_kernel_spmd

F32 = mybir.dt.float32
F16 = mybir.dt.float16
AF = mybir.ActivationFunctionType
ADD = mybir.AluOpType.add
MULT = mybir.AluOpType.mult

B, S, D, H = 2, 2048, 1024, 16
NCORES = 8
HL = 4            # heads per core
DH = D // H       # 64
DLOC = HL * DH    # 256 local feature dims
SBK = 512         # seq block (q)
NSB = S // SBK    # 4
KCH = 128         # k chunk
NCH = S // KCH    # 16


def _ts(i, n):
    return slice(i * n, (i + 1) * n)


def build(debug=False):
    nc = bacc.Bacc(None, target_bir_lowering=False)

    xqT = nc.dram_tensor("xqT", [D, S], F16, kind="ExternalInput")
    xkT = nc.dram_tensor("xkT", [D, S], F16, kind="ExternalInput")
    xvT = nc.dram_tensor("xvT", [D, S], F16, kind="ExternalInput")
    wqT = nc.dram_tensor("wqT", [D, DLOC], F16, kind="ExternalInput")
    wkT = nc.dram_tensor("wkT", [D, DLOC], F16, kind="ExternalInput")
    wvT = nc.dram_tensor("wvT", [D, DLOC], F16, kind="ExternalInput")
    dnT = nc.dram_tensor("dnT", [DLOC, D], F16, kind="ExternalInput")
    outT = nc.dram_tensor("outT", [D, S], F16, kind="ExternalOutput")
    if debug:
        dQT = nc.dram_tensor("dQT", [128, 2, S], F16, kind="ExternalOutput")
        dKT = nc.dram_tensor("dKT", [128, 2, S], F16, kind="ExternalOutput")
        dV = nc.dram_tensor("dV", [128, NCH, HL, DH + 1], F16, kind="ExternalOutput")
        dPT = nc.dram_tensor("dPT", [128, 2, SBK], F16, kind="ExternalOutput")
        dO = nc.dram_tensor("dO", [2, 128, NSB, 128], F32, kind="ExternalOutput")
        dO8 = nc.dram_tensor("dO8", [2, 128, NSB, DH], F16, kind="ExternalOutput")
        dOT = nc.dram_tensor("dOT", [128, 2, NSB, 128], F16, kind="ExternalOutput")

    # lower-tri 0/1 pattern (allowed = k <= q within the diagonal block)
    tri_np = (np.arange(128)[:, None] <= np.arange(128)[None, :]).astype(np.float16)
    tri_c = nc.inline_tensor(tri_np, name="tri01")
    id_c = nc.inline_tensor(np.eye(128, dtype=np.float16), name="id128")

    with tile.TileContext(nc) as tc, ExitStack() as ctx:
        pers = ctx.enter_context(tc.tile_pool(name="pers", bufs=1))
        xpool = ctx.enter_context(tc.tile_pool(name="xpool", bufs=24))
        ptp = ctx.enter_context(tc.tile_pool(name="ptp", bufs=1))  # 16 tags x 1 buf
        o8p = ctx.enter_context(tc.tile_pool(name="o8p", bufs=4))
        otp = ctx.enter_context(tc.tile_pool(name="otp", bufs=2))
        evp = ctx.enter_context(tc.tile_pool(name="evp", bufs=5))
        smallp = ctx.enter_context(tc.tile_pool(name="smallp", bufs=4))
        mmp = ctx.enter_context(tc.tile_pool(name="mmp", bufs=2, space="PSUM"))
        lp = ctx.enter_context(tc.tile_pool(name="lp", bufs=2, space="PSUM"))
        onp = ctx.enter_context(tc.tile_pool(name="onp", bufs=1, space="PSUM"))  # 2 tags x 1 buf

        # ---------- persistent tiles ----------
        wparts = {}
        for wname in ("q", "k", "v"):
            wparts[wname] = [
                pers.tile([128, 2, DLOC], F16, tag=f"w{wname}{kc2}",
                          name=f"w_{wname}_{kc2}")
                for kc2 in range(4)
            ]
        dn_sb = pers.tile([128, 2, D], F16, tag="dn")
        tri_sb = pers.tile([128, 128], F16, tag="tri")
        id_sb = pers.tile([128, 128], F16, tag="id")
        nc.sync.dma_start(out=tri_sb, in_=tri_c[:, :])
        nc.sync.dma_start(out=id_sb, in_=id_c[:, :])

        QT_sb = pers.tile([128, 2, S], F16, tag="QT")
        KT_sb = pers.tile([128, 2, S], F16, tag="KT")
        V_sb = pers.tile([128, NCH, HL, DH + 1], F16, tag="V")
        # ones column (softmax denominator accumulates via PV matmul)
        nc.vector.memset(V_sb[:, :, :, DH:DH + 1], 1.0)

        outT_r = outT.rearrange("(c p) s -> p c s", p=128)

        def load_x(j, js):
            xt = {}
            for xname, src in (("q", xqT), ("k", xkT), ("v", xvT)):
                srcr = src.rearrange("(c p) s -> p c s", p=128)
                tiles = []
                for kc2 in range(4):
                    t = xpool.tile([128, 2, SBK], F16, tag="xt",
                                   name=f"x_{xname}_{j}_{kc2}")
                    nc.sync.dma_start(out=t, in_=srcr[:, _ts(kc2, 2), js])
                    tiles.append(t)
                xt[xname] = tiles
            return xt

        def phase_A(j, js, xt=None):
            # ---------- projections for s-block j ----------
            if xt is None:
                xt = load_x(j, js)

            # evacs on the Activation engine: it is idle during phase A
            for bname, dst in (("q", QT_sb), ("k", KT_sb)):
                for mc in range(2):
                    ps = mmp.tile([128, 512], F32, tag="mm")
                    for kc in range(8):
                        nc.tensor.matmul(
                            ps[:, :],
                            lhsT=wparts[bname][kc // 2][:, kc % 2, _ts(mc, 128)],
                            rhs=xt[bname][kc // 2][:, kc % 2, :],
                            start=(kc == 0), stop=(kc == 7),
                        )
                    nc.scalar.copy(dst[:, mc, js], ps)

            for sc in range(4):
                ps = mmp.tile([128, 512], F32, tag="mm")
                for kc in range(8):
                    nc.tensor.matmul(
                        ps[:, 0:DLOC],
                        lhsT=xt["v"][kc // 2][:, kc % 2, _ts(sc, 128)],
                        rhs=wparts["v"][kc // 2][:, kc % 2, :],
                        start=(kc == 0), stop=(kc == 7),
                    )
                nc.scalar.copy(
                    V_sb[:, j * 4 + sc, :, 0:DH],
                    ps[:, 0:DLOC].rearrange("p (h d) -> p h d", h=HL),
                )

        def phase_B(j, js):
            # ---------- attention + dense for q-block j ----------
            # Emission order is pipelined so the in-order PE queue always has
            # work while DVE runs the recip/normalize/evac chains, and so the
            # Act engine (exp-bound) is fed logits as early as possible.
            nkc = (j + 1) * 4
            OT = otp.tile([128, 2, NSB, 128], F16, tag="ot", name=f"OT_{j}")

            def emit_logits_exp(pc):
                PTs = []
                for kc in range(nkc):
                    off = max(0, kc - 4 * j) * KCH  # causal column trim
                    L = lp.tile([128, 2, SBK], F32, tag="L")
                    for i in range(2):
                        nc.tensor.matmul(
                            L[:, i, off:SBK],
                            lhsT=KT_sb[_ts(i, 64), pc, _ts(kc, KCH)],
                            rhs=QT_sb[_ts(i, 64), pc, j * SBK + off:(j + 1) * SBK],
                            start=True, stop=True,
                            tile_position=(64 * i, 0),
                        )
                    PT = ptp.tile([128, 2, SBK], F16, tag=f"PT{pc}_{kc}",
                                  name=f"PT_{j}_{pc}_{kc}")
                    nc.scalar.activation(
                        out=PT[:, :, off:SBK], in_=L[:, :, off:SBK],
                        func=AF.Exp, scale=0.125)
                    if kc >= 4 * j:
                        # mask the diagonal 128-block (0/1 lower-tri multiply)
                        nc.vector.tensor_tensor(
                            out=PT[:, :, off:off + KCH],
                            in0=PT[:, :, off:off + KCH],
                            in1=tri_sb[:, None, :].broadcast_to([128, 2, KCH]),
                            op=MULT,
                        )
                    if debug and j == 0 and pc == 0 and kc == 0:
                        nc.sync.dma_start(out=dPT[:, :, :], in_=PT)
                    PTs.append(PT)
                return PTs

            PTs = [emit_logits_exp(0), emit_logits_exp(1)]
            Os, O8s = {}, {}

            def emit_pv(pc, i):
                O = onp.tile([128, NSB, 128], F32, tag=f"o{i}",
                             name=f"O_{j}_{pc}_{i}")
                for qs in range(NSB):
                    kmax = 4 * j + qs
                    for kc in range(kmax + 1):
                        nc.tensor.matmul(
                            O[:, qs, 0:DH + 1],
                            lhsT=PTs[pc][kc][:, i, _ts(qs, 128)],
                            rhs=V_sb[:, kc, 2 * pc + i, :],
                            start=(kc == 0), stop=(kc == kmax),
                            skip_group_check=True,
                        )
                Os[pc, i] = O

            def emit_norm(pc, i):
                O = Os[pc, i]
                rc = smallp.tile([128, NSB, 1], F32, tag="rc")
                nc.vector.reciprocal(rc, O[:, :, DH:DH + 1])
                O8 = o8p.tile([128, NSB, DH], F16, tag=f"o8{i}",
                              name=f"O8_{j}_{pc}_{i}")
                nc.vector.tensor_tensor(
                    out=O8, in0=O[:, :, 0:DH],
                    in1=rc.broadcast_to([128, NSB, DH]), op=MULT,
                )
                if debug and j == 0 and pc == 0:
                    ostage = evp.tile([128, NSB, 128], F32, tag="ev",
                                      name=f"ostage_{i}")
                    nc.vector.tensor_copy(ostage, O)
                    nc.sync.dma_start(out=dO[i, :, :, :], in_=ostage)
                    nc.sync.dma_start(out=dO8[i, :, :, :], in_=O8)
                O8s[pc, i] = O8

            def emit_transposes(pc):
                tp = mmp.tile([128, NSB, 128], F16, tag="mm", name=f"tp_{j}_{pc}")
                for i in range(2):
                    for qs in range(NSB):
                        nc.tensor.transpose(
                            tp[_ts(i, 64), qs, :], O8s[pc, i][:, qs, :], id_sb,
                            tile_position=(0, 64 * i),
                        )
                nc.vector.tensor_copy(OT[:, pc, :, :], tp)

            emit_pv(0, 0)
            emit_norm(0, 0)
            emit_pv(0, 1)
            emit_norm(0, 1)
            emit_pv(1, 0)
            emit_norm(1, 0)
            emit_transposes(0)
            emit_pv(1, 1)
            emit_norm(1, 1)
            emit_transposes(1)
            if debug and j == 0:
                nc.sync.dma_start(out=dOT[:, :, :, :], in_=OT)

            for mc in range(8):
                dps = mmp.tile([128, 512], F32, tag="mm")
                for t in range(2):
                    nc.tensor.matmul(
                        dps[:, :],
                        lhsT=dn_sb[:, t, _ts(mc, 128)],
                        rhs=OT[:, t, :, :].rearrange("p a b -> p (a b)"),
                        start=(t == 0), stop=(t == 1),
                    )
                ev = evp.tile([128, 512], F16, tag="ev")
                # alternate evac engine: halves the evac-bound dense stretch
                if mc % 2 == 0:
                    nc.vector.tensor_copy(ev, dps)
                else:
                    nc.scalar.copy(ev, dps)
                nc.sync.dma_start(out=outT_r[:, mc, js], in_=ev)

        # startup: interleave weight-part and first-block x DMAs in
        # consumption order so the first projection matmuls start early
        xt0 = {}
        js0 = _ts(0, SBK)
        for xname, xsrc in (("q", xqT), ("k", xkT), ("v", xvT)):
            srcr = xsrc.rearrange("(c p) s -> p c s", p=128)
            wr = {"q": wqT, "k": wkT, "v": wvT}[xname].rearrange(
                "(c p) m -> p c m", p=128)
            tiles = []
            for kc2 in range(4):
                nc.sync.dma_start(out=wparts[xname][kc2], in_=wr[:, _ts(kc2, 2), :])
                t = xpool.tile([128, 2, SBK], F16, tag="xt",
                               name=f"x_{xname}_0_{kc2}")
                nc.sync.dma_start(out=t, in_=srcr[:, _ts(kc2, 2), js0])
                tiles.append(t)
            xt0[xname] = tiles
        nc.sync.dma_start(
            out=dn_sb, in_=dnT.rearrange("(t p) n -> p t n", p=128))

        xts = {0: xt0}
        for j in range(NSB):
            if j + 1 < NSB:
                xts[j + 1] = load_x(j + 1, _ts(j + 1, SBK))  # prefetch
            phase_A(j, _ts(j, SBK), xt=xts.pop(j))
        if debug:
            nc.sync.dma_start(out=dQT[:, :, :], in_=QT_sb)
            nc.sync.dma_start(out=dKT[:, :, :], in_=KT_sb)
            nc.sync.dma_start(out=dV[:, :, :, :], in_=V_sb)
        for j in range(NSB):
            phase_B(j, _ts(j, SBK))

    nc.finalize()
    return nc


_CACHE = {}


def _get_nc(causal=True, with_bq=False, with_bk=False, with_bv=False):
    key = (causal, with_bq, with_bk, with_bv)
    if key not in _CACHE:
        assert causal and not (with_bq or with_bk or with_bv)
        _CACHE[key] = build()
    return _CACHE[key]


def _numpy_fallback(query, key_, value, mask, wq_w, wq_b, wk_w, wk_b, wv_w,
                    wv_b, dense_w, dense_b):
    out = np.empty((B, S, D), np.float32)
    m4 = np.asarray(mask, np.float32).reshape(-1, S, S)
    for b in range(B):
        q = (query[b] @ wq_w.T + wq_b).reshape(S, H, DH).transpose(1, 0, 2)
        k = (key_[b] @ wk_w.T + wk_b).reshape(S, H, DH).transpose(1, 0, 2)
        v = (value[b] @ wv_w.T + wv_b).reshape(S, H, DH).transpose(1, 0, 2)
        mb = m4[min(b, m4.shape[0] - 1)]
        o = np.empty((H, S, DH), np.float32)
        for h in range(H):
            lg = (q[h] @ k[h].T) / np.sqrt(np.float32(DH)) + mb * np.float32(-1e9)
            lg -= lg.max(-1, keepdims=True)
            p = np.exp(lg)
            p /= p.sum(-1, keepdims=True)
            o[h] = p @ v[h]
        out[b] = o.transpose(1, 0, 2).reshape(S, D) @ dense_w.T + dense_b
    return out


def _prep_in_maps(query, key_, value, wq_w, wk_w, wv_w, dense_w):
    xT = {}
    for b in range(B):
        xT[b] = (
            np.ascontiguousarray(query[b].T).astype(np.float16),
            np.ascontiguousarray(key_[b].T).astype(np.float16),
            np.ascontiguousarray(value[b].T).astype(np.float16),
        )
    in_maps = []
    for c in range(NCORES):
        b, g = divmod(c, 4)
        sl = _ts(g, DLOC)
        in_maps.append({
            "xqT": xT[b][0], "xkT": xT[b][1], "xvT": xT[b][2],
            "wqT": np.ascontiguousarray(wq_w[sl].T).astype(np.float16),
            "wkT": np.ascontiguousarray(wk_w[sl].T).astype(np.float16),
            "wvT": np.ascontiguousarray(wv_w[sl].T).astype(np.float16),
            "dnT": np.ascontiguousarray(dense_w[:, sl].T).astype(np.float16),
        })
    return in_maps


def kernel(query, key_, value, mask, wq_w, wq_b, wk_w, wk_b, wv_w, wv_b,
           dense_w, dense_b, _profile_kw=None):
    query = np.asarray(query, np.float32)
    key_ = np.asarray(key_, np.float32)
    value = np.asarray(value, np.float32)
    mask2d = np.asarray(mask, np.float32).reshape(S, S)
    wq_w = np.asarray(wq_w, np.float32)
    wk_w = np.asarray(wk_w, np.float32)
    wv_w = np.asarray(wv_w, np.float32)
    dense_w = np.asarray(dense_w, np.float32)
    dense_b = np.asarray(dense_b, np.float32)

    causal = bool(np.array_equal(mask2d, np.triu(np.ones((S, S), np.float32), k=1)))
    if not causal or np.any(wq_b) or np.any(wk_b) or np.any(wv_b):
        out = _numpy_fallback(query, key_, value, mask, wq_w, wq_b, wk_w,
                              wk_b, wv_w, wv_b, dense_w, dense_b)
        return (out, None) if _profile_kw else out

    in_maps = _prep_in_maps(query, key_, value, wq_w, wk_w, wv_w, dense_w)
    nc = _get_nc(True, False, False, False)
    res = run_bass_kernel_spmd(nc, in_maps, core_ids=list(range(NCORES)),
                               **(_profile_kw or {}))

    out = np.empty((B, S, D), np.float32)
    for b in range(B):
        acc = res.results[4 * b]["outT"].astype(np.float32)
        for g in range(1, 4):
            acc = acc + res.results[4 * b + g]["outT"].astype(np.float32)
        out[b] = acc.T + dense_b[None, :]
    if _profile_kw:
        return out, res
    return out


# revision 21
# speedup vs baseline: 1.1626x; 1.0335x over previous
"""Multi-head attention (B=2, S=2048, D=1024, H=16) on 8 TRN2 NeuronCores.

Sharding: batch x head-group. Core c handles batch c//4 and heads
[4*(c%4), 4*(c%4)+4). Each core computes its heads' Q/K/V projections
(column-parallel), causal attention, and a row-parallel partial of the
output projection. The host sums the 4 partials per batch and adds dense_b.

All matmul operands are fp16 (fp32 PSUM accumulation). On-core dataflow:
  QT/KT [128, 2, S] fp16: partition = head-pair-local feature (2 heads x 64),
    second dim = head pair (pc). V [128p=seq, chunk, head, 65] fp16 with a
    ones column (col 64) for the softmax denominator.
  per q-block j (512 wide), head pair pc, k-chunk kc (128 wide):
    L[:, i, off:] = KT_chunk.T @ QT_block   (2 heads row-packed in the PE,
      columns below the causal diagonal skipped)
    PT = exp(0.125 * L)  fp16  (ScalarE only does exp; diagonal 128-block
      masked multiplicatively with a 0/1 lower-tri pattern on DVE)
    per head i, per q-sub qs (128 wide, qs >= chunk diagonal): natural-
      orientation PV: O[i][:, qs, 0:65] += PT[:, i, qs-cols].T @ V_aug
      (full 128x128 PE payload, 65 streamed rows)
  per head: recip = 1/O[:, :, 64] (DVE); O8 = O * recip (fused PSUM evac);
    OT = PE-transpose(O8) per q-sub into a shared PSUM tile; one DVE copy
    to OT_sb [128, 2, 4, 128] fp16 (partition = dense contraction dim).
  dense: outT[mc*128:+128, q-block] = sum_t dnT[:, t, mc].T @ OT_sb[:, t]
    evacuated on the Pool engine (fp16) and DMA'd out.
"""

import numpy as np
from contextlib import ExitStack

import concourse.tile as tile
from concourse import bacc, mybir
from concourse.bass_utils import run_bass_kernel_spmd

F32 = mybir.dt.float32
F16 = mybir.dt.float16
I32 = mybir.dt.int32
AF = mybir.ActivationFunctionType
ADD = mybir.AluOpType.add
MULT = mybir.AluOpType.mult

B, S, D, H = 2, 2048, 1024, 16
NCORES = 8
HL = 4            # heads per core
DH = D // H       # 64
DLOC = HL * DH    # 256 local feature dims
SBK = 512         # seq block (q)
NSB = S // SBK    # 4
KCH = 128         # k chunk
NCH = S // KCH    # 16
# Schraudolph exp constants (0.125 softmax scale folded in): exp(0.125*x)
# ~= bitcast_f32(int32(x*EA + EB)); ~3% max relative error
EA = float(2 ** 23 / np.log(2) * 0.125)
EB = float(127 * 2 ** 23 - 0.043677448 * 2 ** 23)


def _ts(i, n):
    return slice(i * n, (i + 1) * n)


def build(debug=False):
    nc = bacc.Bacc(None, target_bir_lowering=False)

    xqT = nc.dram_tensor("xqT", [D, S], F16, kind="ExternalInput")
    xkT = nc.dram_tensor("xkT", [D, S], F16, kind="ExternalInput")
    xvT = nc.dram_tensor("xvT", [D, S], F16, kind="ExternalInput")
    wqT = nc.dram_tensor("wqT", [D, DLOC], F16, kind="ExternalInput")
    wkT = nc.dram_tensor("wkT", [D, DLOC], F16, kind="ExternalInput")
    wvT = nc.dram_tensor("wvT", [D, DLOC], F16, kind="ExternalInput")
    dnT = nc.dram_tensor("dnT", [DLOC, D], F16, kind="ExternalInput")
    outT = nc.dram_tensor("outT", [D, S], F16, kind="ExternalOutput")
    if debug:
        dQT = nc.dram_tensor("dQT", [128, 2, S], F16, kind="ExternalOutput")
        dKT = nc.dram_tensor("dKT", [128, 2, S], F16, kind="ExternalOutput")
        dV = nc.dram_tensor("dV", [128, NCH, HL, DH + 1], F16, kind="ExternalOutput")
        dPT = nc.dram_tensor("dPT", [128, 2, SBK], F16, kind="ExternalOutput")
        dO = nc.dram_tensor("dO", [2, 128, NSB, 128], F32, kind="ExternalOutput")
        dO8 = nc.dram_tensor("dO8", [2, 128, NSB, DH], F16, kind="ExternalOutput")
        dOT = nc.dram_tensor("dOT", [128, 2, NSB, 128], F16, kind="ExternalOutput")

    # lower-tri 0/1 pattern (allowed = k <= q within the diagonal block)
    tri_np = (np.arange(128)[:, None] <= np.arange(128)[None, :]).astype(np.float16)
    tri_c = nc.inline_tensor(tri_np, name="tri01")
    id_c = nc.inline_tensor(np.eye(128, dtype=np.float16), name="id128")

    with tile.TileContext(nc) as tc, ExitStack() as ctx:
        pers = ctx.enter_context(tc.tile_pool(name="pers", bufs=1))
        xpool = ctx.enter_context(tc.tile_pool(name="xpool", bufs=24))
        ptp = ctx.enter_context(tc.tile_pool(name="ptp", bufs=1))  # 16 tags x 1 buf
        o8p = ctx.enter_context(tc.tile_pool(name="o8p", bufs=4))
        otp = ctx.enter_context(tc.tile_pool(name="otp", bufs=2))
        evp = ctx.enter_context(tc.tile_pool(name="evp", bufs=5))
        smallp = ctx.enter_context(tc.tile_pool(name="smallp", bufs=4))
        schp = ctx.enter_context(tc.tile_pool(name="schp", bufs=2))
        mmp = ctx.enter_context(tc.tile_pool(name="mmp", bufs=2, space="PSUM"))
        lp = ctx.enter_context(tc.tile_pool(name="lp", bufs=2, space="PSUM"))
        onp = ctx.enter_context(tc.tile_pool(name="onp", bufs=1, space="PSUM"))  # 2 tags x 1 buf

        # ---------- persistent tiles ----------
        wparts = {}
        for wname in ("q", "k", "v"):
            wparts[wname] = [
                pers.tile([128, 2, DLOC], F16, tag=f"w{wname}{kc2}",
                          name=f"w_{wname}_{kc2}")
                for kc2 in range(4)
            ]
        dn_sb = pers.tile([128, 2, D], F16, tag="dn")
        tri_sb = pers.tile([128, 128], F16, tag="tri")
        id_sb = pers.tile([128, 128], F16, tag="id")

        QT_sb = pers.tile([128, 2, S], F16, tag="QT")
        KT_sb = pers.tile([128, 2, S], F16, tag="KT")
        V_sb = pers.tile([128, NCH, HL, DH + 1], F16, tag="V")
        # ones column (softmax denominator accumulates via PV matmul)
        nc.vector.memset(V_sb[:, :, :, DH:DH + 1], 1.0)

        outT_r = outT.rearrange("(c p) s -> p c s", p=128)

        def load_x(j, js):
            xt = {}
            for xname, src in (("q", xqT), ("k", xkT), ("v", xvT)):
                srcr = src.rearrange("(c p) s -> p c s", p=128)
                tiles = []
                for kc2 in range(4):
                    t = xpool.tile([128, 2, SBK], F16, tag="xt",
                                   name=f"x_{xname}_{j}_{kc2}")
                    nc.sync.dma_start(out=t, in_=srcr[:, _ts(kc2, 2), js])
                    tiles.append(t)
                xt[xname] = tiles
            return xt

        def phase_A(j, js, xt=None):
            # ---------- projections for s-block j ----------
            if xt is None:
                xt = load_x(j, js)

            # evacs on the Activation engine: it is idle during phase A
            for bname, dst in (("q", QT_sb), ("k", KT_sb)):
                for mc in range(2):
                    ps = mmp.tile([128, 512], F32, tag="mm")
                    for kc in range(8):
                        nc.tensor.matmul(
                            ps[:, :],
                            lhsT=wparts[bname][kc // 2][:, kc % 2, _ts(mc, 128)],
                            rhs=xt[bname][kc // 2][:, kc % 2, :],
                            start=(kc == 0), stop=(kc == 7),
                        )
                    nc.scalar.copy(dst[:, mc, js], ps)

            for sc in range(4):
                ps = mmp.tile([128, 512], F32, tag="mm")
                for kc in range(8):
                    nc.tensor.matmul(
                        ps[:, 0:DLOC],
                        lhsT=xt["v"][kc // 2][:, kc % 2, _ts(sc, 128)],
                        rhs=wparts["v"][kc // 2][:, kc % 2, :],
                        start=(kc == 0), stop=(kc == 7),
                    )
                nc.scalar.copy(
                    V_sb[:, j * 4 + sc, :, 0:DH],
                    ps[:, 0:DLOC].rearrange("p (h d) -> p h d", h=HL),
                )

        def phase_B(j, js):
            # ---------- attention + dense for q-block j ----------
            # Emission order is pipelined so the in-order PE queue always has
            # work while DVE runs the recip/normalize/evac chains, and so the
            # Act engine (exp-bound) is fed logits as early as possible.
            nkc = (j + 1) * 4
            OT = otp.tile([128, 2, NSB, 128], F16, tag="ot", name=f"OT_{j}")

            def emit_logits_exp(pc):
                PTs = []
                for kc in range(nkc):
                    off = max(0, kc - 4 * j) * KCH  # causal column trim
                    L = lp.tile([128, 2, SBK], F32, tag="L")
                    for i in range(2):
                        nc.tensor.matmul(
                            L[:, i, off:SBK],
                            lhsT=KT_sb[_ts(i, 64), pc, _ts(kc, KCH)],
                            rhs=QT_sb[_ts(i, 64), pc, j * SBK + off:(j + 1) * SBK],
                            start=True, stop=True,
                            tile_position=(64 * i, 0),
                        )
                    PT = ptp.tile([128, 2, SBK], F16, tag=f"PT{pc}_{kc}",
                                  name=f"PT_{j}_{pc}_{kc}")
                    if kc < 4 * j and kc % 4 == 1:
                        # offload this full chunk's exp to DVE (Schraudolph
                        # bit-trick): Act is the phase-B bottleneck engine
                        T = schp.tile([128, 2, SBK], I32, tag="sch")
                        nc.vector.tensor_scalar(
                            out=T, in0=L, scalar1=EA, scalar2=EB,
                            op0=MULT, op1=ADD)
                        nc.vector.tensor_copy(PT, T.bitcast(F32))
                    else:
                        nc.scalar.activation(
                            out=PT[:, :, off:SBK], in_=L[:, :, off:SBK],
                            func=AF.Exp, scale=0.125)
                    if kc >= 4 * j:
                        # mask the diagonal 128-block (0/1 lower-tri multiply)
                        # on the otherwise-idle Pool engine (SBUF-only op)
                        nc.gpsimd.tensor_tensor(
                            out=PT[:, :, off:off + KCH],
                            in0=PT[:, :, off:off + KCH],
                            in1=tri_sb[:, None, :].broadcast_to([128, 2, KCH]),
                            op=MULT,
                        )
                    if debug and j == 0 and pc == 0 and kc == 0:
                        nc.sync.dma_start(out=dPT[:, :, :], in_=PT)
                    PTs.append(PT)
                return PTs

            PTs = [emit_logits_exp(0), emit_logits_exp(1)]
            Os, O8s = {}, {}

            def emit_pv(pc, i):
                O = onp.tile([128, NSB, 128], F32, tag=f"o{i}",
                             name=f"O_{j}_{pc}_{i}")
                for qs in range(NSB):
                    kmax = 4 * j + qs
                    for kc in range(kmax + 1):
                        nc.tensor.matmul(
                            O[:, qs, 0:DH + 1],
                            lhsT=PTs[pc][kc][:, i, _ts(qs, 128)],
                            rhs=V_sb[:, kc, 2 * pc + i, :],
                            start=(kc == 0), stop=(kc == kmax),
                            skip_group_check=True,
                        )
                Os[pc, i] = O

            def emit_norm(pc, i):
                O = Os[pc, i]
                rc = smallp.tile([128, NSB, 1], F32, tag="rc")
                nc.vector.reciprocal(rc, O[:, :, DH:DH + 1])
                O8 = o8p.tile([128, NSB, DH], F16, tag=f"o8{i}",
                              name=f"O8_{j}_{pc}_{i}")
                nc.vector.tensor_tensor(
                    out=O8, in0=O[:, :, 0:DH],
                    in1=rc.broadcast_to([128, NSB, DH]), op=MULT,
                )
                if debug and j == 0 and pc == 0:
                    ostage = evp.tile([128, NSB, 128], F32, tag="ev",
                                      name=f"ostage_{i}")
                    nc.vector.tensor_copy(ostage, O)
                    nc.sync.dma_start(out=dO[i, :, :, :], in_=ostage)
                    nc.sync.dma_start(out=dO8[i, :, :, :], in_=O8)
                O8s[pc, i] = O8

            def emit_transposes(pc):
                tp = mmp.tile([128, NSB, 128], F16, tag="mm", name=f"tp_{j}_{pc}")
                for i in range(2):
                    for qs in range(NSB):
                        nc.tensor.transpose(
                            tp[_ts(i, 64), qs, :], O8s[pc, i][:, qs, :], id_sb,
                            tile_position=(0, 64 * i),
                        )
                nc.vector.tensor_copy(OT[:, pc, :, :], tp)

            emit_pv(0, 0)
            emit_norm(0, 0)
            emit_pv(0, 1)
            emit_norm(0, 1)
            emit_pv(1, 0)
            emit_norm(1, 0)
            emit_transposes(0)
            emit_pv(1, 1)
            emit_norm(1, 1)
            emit_transposes(1)
            if debug and j == 0:
                nc.sync.dma_start(out=dOT[:, :, :, :], in_=OT)

            for mc in range(8):
                dps = mmp.tile([128, 512], F32, tag="mm")
                for t in range(2):
                    nc.tensor.matmul(
                        dps[:, :],
                        lhsT=dn_sb[:, t, _ts(mc, 128)],
                        rhs=OT[:, t, :, :].rearrange("p a b -> p (a b)"),
                        start=(t == 0), stop=(t == 1),
                    )
                ev = evp.tile([128, 512], F16, tag="ev")
                # last block: alternate evac engine to shorten the drain tail
                if j == NSB - 1 and mc % 2 == 1:
                    nc.scalar.copy(ev, dps)
                else:
                    nc.vector.tensor_copy(ev, dps)
                nc.sync.dma_start(out=outT_r[:, mc, js], in_=ev)

        # startup: interleave weight-part and first-block x DMAs in
        # consumption order so the first projection matmuls start early
        xt0 = {}
        js0 = _ts(0, SBK)
        for xname, xsrc in (("q", xqT), ("k", xkT), ("v", xvT)):
            srcr = xsrc.rearrange("(c p) s -> p c s", p=128)
            wr = {"q": wqT, "k": wkT, "v": wvT}[xname].rearrange(
                "(c p) m -> p c m", p=128)
            tiles = []
            for kc2 in range(4):
                nc.sync.dma_start(out=wparts[xname][kc2], in_=wr[:, _ts(kc2, 2), :])
                t = xpool.tile([128, 2, SBK], F16, tag="xt",
                               name=f"x_{xname}_0_{kc2}")
                nc.sync.dma_start(out=t, in_=srcr[:, _ts(kc2, 2), js0])
                tiles.append(t)
            xt0[xname] = tiles
        nc.sync.dma_start(
            out=dn_sb, in_=dnT.rearrange("(t p) n -> p t n", p=128))
        nc.sync.dma_start(out=tri_sb, in_=tri_c[:, :])
        nc.sync.dma_start(out=id_sb, in_=id_c[:, :])

        xts = {0: xt0}
        for j in range(NSB):
            if j + 1 < NSB:
                xts[j + 1] = load_x(j + 1, _ts(j + 1, SBK))  # prefetch
            phase_A(j, _ts(j, SBK), xt=xts.pop(j))
        if debug:
            nc.sync.dma_start(out=dQT[:, :, :], in_=QT_sb)
            nc.sync.dma_start(out=dKT[:, :, :], in_=KT_sb)
            nc.sync.dma_start(out=dV[:, :, :, :], in_=V_sb)
        for j in range(NSB):
            phase_B(j, _ts(j, SBK))

    nc.finalize()
    return nc


_CACHE = {}


def _get_nc(causal=True, with_bq=False, with_bk=False, with_bv=False):
    key = (causal, with_bq, with_bk, with_bv)
    if key not in _CACHE:
        assert causal and not (with_bq or with_bk or with_bv)
        _CACHE[key] = build()
    return _CACHE[key]


def _numpy_fallback(query, key_, value, mask, wq_w, wq_b, wk_w, wk_b, wv_w,
                    wv_b, dense_w, dense_b):
    out = np.empty((B, S, D), np.float32)
    m4 = np.asarray(mask, np.float32).reshape(-1, S, S)
    for b in range(B):
        q = (query[b] @ wq_w.T + wq_b).reshape(S, H, DH).transpose(1, 0, 2)
        k = (key_[b] @ wk_w.T + wk_b).reshape(S, H, DH).transpose(1, 0, 2)
        v = (value[b] @ wv_w.T + wv_b).reshape(S, H, DH).transpose(1, 0, 2)
        mb = m4[min(b, m4.shape[0] - 1)]
        o = np.empty((H, S, DH), np.float32)
        for h in range(H):
            lg = (q[h] @ k[h].T) / np.sqrt(np.float32(DH)) + mb * np.float32(-1e9)
            lg -= lg.max(-1, keepdims=True)
            p = np.exp(lg)
            p /= p.sum(-1, keepdims=True)
            o[h] = p @ v[h]
        out[b] = o.transpose(1, 0, 2).reshape(S, D) @ dense_w.T + dense_b
    return out


def _prep_in_maps(query, key_, value, wq_w, wk_w, wv_w, dense_w):
    xT = {}
    for b in range(B):
        xT[b] = (
            np.ascontiguousarray(query[b].T).astype(np.float16),
            np.ascontiguousarray(key_[b].T).astype(np.float16),
            np.ascontiguousarray(value[b].T).astype(np.float16),
        )
    in_maps = []
    for c in range(NCORES):
        b, g = divmod(c, 4)
        sl = _ts(g, DLOC)
        in_maps.append({
            "xqT": xT[b][0], "xkT": xT[b][1], "xvT": xT[b][2],
            "wqT": np.ascontiguousarray(wq_w[sl].T).astype(np.float16),
            "wkT": np.ascontiguousarray(wk_w[sl].T).astype(np.float16),
            "wvT": np.ascontiguousarray(wv_w[sl].T).astype(np.float16),
            "dnT": np.ascontiguousarray(dense_w[:, sl].T).astype(np.float16),
        })
    return in_maps


def kernel(query, key_, value, mask, wq_w, wq_b, wk_w, wk_b, wv_w, wv_b,
           dense_w, dense_b, _profile_kw=None):
    query = np.asarray(query, np.float32)
    key_ = np.asarray(key_, np.float32)
    value = np.asarray(value, np.float32)
    mask2d = np.asarray(mask, np.float32).reshape(S, S)
    wq_w = np.asarray(wq_w, np.float32)
    wk_w = np.asarray(wk_w, np.float32)
    wv_w = np.asarray(wv_w, np.float32)
    dense_w = np.asarray(dense_w, np.float32)
    dense_b = np.asarray(dense_b, np.float32)

    causal = bool(np.array_equal(mask2d, np.triu(np.ones((S, S), np.float32), k=1)))
    if not causal or np.any(wq_b) or np.any(wk_b) or np.any(wv_b):
        out = _numpy_fallback(query, key_, value, mask, wq_w, wq_b, wk_w,
                              wk_b, wv_w, wv_b, dense_w, dense_b)
        return (out, None) if _profile_kw else out

    in_maps = _prep_in_maps(query, key_, value, wq_w, wk_w, wv_w, dense_w)
    nc = _get_nc(True, False, False, False)
    res = run_bass_kernel_spmd(nc, in_maps, core_ids=list(range(NCORES)),
                               **(_profile_kw or {}))

    out = np.empty((B, S, D), np.float32)
    for b in range(B):
        acc = res.results[4 * b]["outT"].astype(np.float32)
        for g in range(1, 4):
            acc = acc + res.results[4 * b + g]["outT"].astype(np.float32)
        out[b] = acc.T + dense_b[None, :]
    if _profile_kw:
        return out, res
    return out


# revision 23
# speedup vs baseline: 1.2195x; 1.0489x over previous
"""Multi-head attention (B=2, S=2048, D=1024, H=16) on 8 TRN2 NeuronCores.

Sharding: batch x head-group. Core c handles batch c//4 and heads
[4*(c%4), 4*(c%4)+4). Each core computes its heads' Q/K/V projections
(column-parallel), causal attention, and a row-parallel partial of the
output projection. The host sums the 4 partials per batch and adds dense_b.

All matmul operands are fp16 (fp32 PSUM accumulation). On-core dataflow:
  QT/KT [128, 2, S] fp16: partition = head-pair-local feature (2 heads x 64),
    second dim = head pair (pc). V [128p=seq, chunk, head, 65] fp16 with a
    ones column (col 64) for the softmax denominator.
  per q-block j (512 wide), head pair pc, k-chunk kc (128 wide):
    L[:, i, off:] = KT_chunk.T @ QT_block   (2 heads row-packed in the PE,
      columns below the causal diagonal skipped)
    PT = exp(0.125 * L)  fp16  (ScalarE only does exp; diagonal 128-block
      masked multiplicatively with a 0/1 lower-tri pattern on DVE)
    per head i, per q-sub qs (128 wide, qs >= chunk diagonal): natural-
      orientation PV: O[i][:, qs, 0:65] += PT[:, i, qs-cols].T @ V_aug
      (full 128x128 PE payload, 65 streamed rows)
  per head: recip = 1/O[:, :, 64] (DVE); O8 = O * recip (fused PSUM evac);
    OT = PE-transpose(O8) per q-sub into a shared PSUM tile; one DVE copy
    to OT_sb [128, 2, 4, 128] fp16 (partition = dense contraction dim).
  dense: outT[mc*128:+128, q-block] = sum_t dnT[:, t, mc].T @ OT_sb[:, t]
    evacuated on the Pool engine (fp16) and DMA'd out.
"""

import numpy as np
from contextlib import ExitStack

import concourse.tile as tile
from concourse import bacc, mybir
from concourse.bass_utils import run_bass_kernel_spmd

F32 = mybir.dt.float32
F16 = mybir.dt.float16
I32 = mybir.dt.int32
AF = mybir.ActivationFunctionType
ADD = mybir.AluOpType.add
MULT = mybir.AluOpType.mult

B, S, D, H = 2, 2048, 1024, 16
NCORES = 8
HL = 4            # heads per core
DH = D // H       # 64
DLOC = HL * DH    # 256 local feature dims
SBK = 512         # seq block (q)
NSB = S // SBK    # 4
KCH = 128         # k chunk
NCH = S // KCH    # 16
# Schraudolph exp constants (0.125 softmax scale folded in): exp(0.125*x)
# ~= bitcast_f32(int32(x*EA + EB)); ~3% max relative error
EA = float(2 ** 23 / np.log(2) * 0.125)
EB = float(127 * 2 ** 23 - 0.043677448 * 2 ** 23)


def _ts(i, n):
    return slice(i * n, (i + 1) * n)


def build(debug=False):
    nc = bacc.Bacc(None, target_bir_lowering=False)

    xqT = nc.dram_tensor("xqT", [D, S], F16, kind="ExternalInput")
    xkT = nc.dram_tensor("xkT", [D, S], F16, kind="ExternalInput")
    xvT = nc.dram_tensor("xvT", [D, S], F16, kind="ExternalInput")
    wqT = nc.dram_tensor("wqT", [D, DLOC], F16, kind="ExternalInput")
    wkT = nc.dram_tensor("wkT", [D, DLOC], F16, kind="ExternalInput")
    wvT = nc.dram_tensor("wvT", [D, DLOC], F16, kind="ExternalInput")
    dnT = nc.dram_tensor("dnT", [DLOC, D], F16, kind="ExternalInput")
    outT = nc.dram_tensor("outT", [D, S], F16, kind="ExternalOutput")
    if debug:
        dQT = nc.dram_tensor("dQT", [128, 2, S], F16, kind="ExternalOutput")
        dKT = nc.dram_tensor("dKT", [128, 2, S], F16, kind="ExternalOutput")
        dV = nc.dram_tensor("dV", [128, NCH, HL, DH + 1], F16, kind="ExternalOutput")
        dPT = nc.dram_tensor("dPT", [128, 2, SBK], F16, kind="ExternalOutput")
        dO = nc.dram_tensor("dO", [2, 128, NSB, 128], F32, kind="ExternalOutput")
        dO8 = nc.dram_tensor("dO8", [2, 128, NSB, DH], F16, kind="ExternalOutput")
        dOT = nc.dram_tensor("dOT", [128, 2, NSB, 128], F16, kind="ExternalOutput")

    # lower-tri 0/1 pattern (allowed = k <= q within the diagonal block)
    tri_np = (np.arange(128)[:, None] <= np.arange(128)[None, :]).astype(np.float16)
    tri_c = nc.inline_tensor(tri_np, name="tri01")
    id_c = nc.inline_tensor(np.eye(128, dtype=np.float16), name="id128")

    with tile.TileContext(nc) as tc, ExitStack() as ctx:
        pers = ctx.enter_context(tc.tile_pool(name="pers", bufs=1))
        xpool = ctx.enter_context(tc.tile_pool(name="xpool", bufs=24))
        ptp = ctx.enter_context(tc.tile_pool(name="ptp", bufs=1))  # 16 tags x 1 buf
        o8p = ctx.enter_context(tc.tile_pool(name="o8p", bufs=4))
        otp = ctx.enter_context(tc.tile_pool(name="otp", bufs=2))
        evp = ctx.enter_context(tc.tile_pool(name="evp", bufs=5))
        smallp = ctx.enter_context(tc.tile_pool(name="smallp", bufs=4))
        schp = ctx.enter_context(tc.tile_pool(name="schp", bufs=2))
        mmp = ctx.enter_context(tc.tile_pool(name="mmp", bufs=2, space="PSUM"))
        lp = ctx.enter_context(tc.tile_pool(name="lp", bufs=2, space="PSUM"))
        onp = ctx.enter_context(tc.tile_pool(name="onp", bufs=1, space="PSUM"))  # 2 tags x 1 buf

        # ---------- persistent tiles ----------
        wparts = {}
        for wname in ("q", "k", "v"):
            wparts[wname] = [
                pers.tile([128, 2, DLOC], F16, tag=f"w{wname}{kc2}",
                          name=f"w_{wname}_{kc2}")
                for kc2 in range(4)
            ]
        dn_sb = pers.tile([128, 2, D], F16, tag="dn")
        tri_sb = pers.tile([128, 128], F16, tag="tri")
        id_sb = pers.tile([128, 128], F16, tag="id")

        QT_sb = pers.tile([128, 2, S], F16, tag="QT")
        KT_sb = pers.tile([128, 2, S], F16, tag="KT")
        V_sb = pers.tile([128, NCH, HL, DH + 1], F16, tag="V")
        # ones column (softmax denominator accumulates via PV matmul)
        nc.vector.memset(V_sb[:, :, :, DH:DH + 1], 1.0)

        outT_r = outT.rearrange("(c p) s -> p c s", p=128)

        def load_x(j, js):
            xt = {}
            for xname, src in (("q", xqT), ("k", xkT), ("v", xvT)):
                srcr = src.rearrange("(c p) s -> p c s", p=128)
                tiles = []
                for kc2 in range(4):
                    t = xpool.tile([128, 2, SBK], F16, tag="xt",
                                   name=f"x_{xname}_{j}_{kc2}")
                    nc.sync.dma_start(out=t, in_=srcr[:, _ts(kc2, 2), js])
                    tiles.append(t)
                xt[xname] = tiles
            return xt

        def phase_A(j, js, xt=None):
            # ---------- projections for s-block j ----------
            if xt is None:
                xt = load_x(j, js)

            # evacs on the Activation engine: it is idle during phase A
            for bname, dst in (("q", QT_sb), ("k", KT_sb)):
                for mc in range(2):
                    ps = mmp.tile([128, 512], F32, tag="mm")
                    for kc in range(8):
                        nc.tensor.matmul(
                            ps[:, :],
                            lhsT=wparts[bname][kc // 2][:, kc % 2, _ts(mc, 128)],
                            rhs=xt[bname][kc // 2][:, kc % 2, :],
                            start=(kc == 0), stop=(kc == 7),
                        )
                    nc.scalar.copy(dst[:, mc, js], ps)

            for sc in range(4):
                ps = mmp.tile([128, 512], F32, tag="mm")
                for kc in range(8):
                    nc.tensor.matmul(
                        ps[:, 0:DLOC],
                        lhsT=xt["v"][kc // 2][:, kc % 2, _ts(sc, 128)],
                        rhs=wparts["v"][kc // 2][:, kc % 2, :],
                        start=(kc == 0), stop=(kc == 7),
                    )
                nc.scalar.copy(
                    V_sb[:, j * 4 + sc, :, 0:DH],
                    ps[:, 0:DLOC].rearrange("p (h d) -> p h d", h=HL),
                )

        def phase_LB(j, js):
            # ---------- logits + exp for q-block j (PE fast / Act slow) ----
            nkc = (j + 1) * 4

            def emit_logits_exp(pc):
                PTs = []
                for kc in range(nkc):
                    off = max(0, kc - 4 * j) * KCH  # causal column trim
                    L = lp.tile([128, 2, SBK], F32, tag="L")
                    for i in range(2):
                        nc.tensor.matmul(
                            L[:, i, off:SBK],
                            lhsT=KT_sb[_ts(i, 64), pc, _ts(kc, KCH)],
                            rhs=QT_sb[_ts(i, 64), pc, j * SBK + off:(j + 1) * SBK],
                            start=True, stop=True,
                            tile_position=(64 * i, 0),
                        )
                    PT = ptp.tile([128, 2, SBK], F16, tag=f"PT{pc}_{kc}",
                                  name=f"PT_{j}_{pc}_{kc}")
                    if kc < 4 * j and kc % 4 == 1:
                        # offload this full chunk's exp to DVE (Schraudolph
                        # bit-trick): Act is the phase-B bottleneck engine
                        T = schp.tile([128, 2, SBK], I32, tag="sch")
                        nc.vector.tensor_scalar(
                            out=T, in0=L, scalar1=EA, scalar2=EB,
                            op0=MULT, op1=ADD)
                        nc.vector.tensor_copy(PT, T.bitcast(F32))
                    else:
                        nc.scalar.activation(
                            out=PT[:, :, off:SBK], in_=L[:, :, off:SBK],
                            func=AF.Exp, scale=0.125)
                    if kc >= 4 * j:
                        # mask the diagonal 128-block (0/1 lower-tri multiply)
                        # on the otherwise-idle Pool engine (SBUF-only op)
                        nc.gpsimd.tensor_tensor(
                            out=PT[:, :, off:off + KCH],
                            in0=PT[:, :, off:off + KCH],
                            in1=tri_sb[:, None, :].broadcast_to([128, 2, KCH]),
                            op=MULT,
                        )
                    if debug and j == 0 and pc == 0 and kc == 0:
                        nc.sync.dma_start(out=dPT[:, :, :], in_=PT)
                    PTs.append(PT)
                return PTs

            return [emit_logits_exp(0), emit_logits_exp(1)]

        def phase_PB(j, js, PTs):
            # ---------- PV + normalize + transpose + dense (PE-heavy) ------
            OT = otp.tile([128, 2, NSB, 128], F16, tag="ot", name=f"OT_{j}")
            Os, O8s = {}, {}

            def emit_pv(pc, i):
                O = onp.tile([128, NSB, 128], F32, tag=f"o{i}",
                             name=f"O_{j}_{pc}_{i}")
                for qs in range(NSB):
                    kmax = 4 * j + qs
                    for kc in range(kmax + 1):
                        nc.tensor.matmul(
                            O[:, qs, 0:DH + 1],
                            lhsT=PTs[pc][kc][:, i, _ts(qs, 128)],
                            rhs=V_sb[:, kc, 2 * pc + i, :],
                            start=(kc == 0), stop=(kc == kmax),
                            skip_group_check=True,
                        )
                Os[pc, i] = O

            def emit_norm(pc, i):
                O = Os[pc, i]
                rc = smallp.tile([128, NSB, 1], F32, tag="rc")
                nc.vector.reciprocal(rc, O[:, :, DH:DH + 1])
                O8 = o8p.tile([128, NSB, DH], F16, tag=f"o8{i}",
                              name=f"O8_{j}_{pc}_{i}")
                nc.vector.tensor_tensor(
                    out=O8, in0=O[:, :, 0:DH],
                    in1=rc.broadcast_to([128, NSB, DH]), op=MULT,
                )
                if debug and j == 0 and pc == 0:
                    ostage = evp.tile([128, NSB, 128], F32, tag="ev",
                                      name=f"ostage_{i}")
                    nc.vector.tensor_copy(ostage, O)
                    nc.sync.dma_start(out=dO[i, :, :, :], in_=ostage)
                    nc.sync.dma_start(out=dO8[i, :, :, :], in_=O8)
                O8s[pc, i] = O8

            def emit_transposes(pc):
                tp = mmp.tile([128, NSB, 128], F16, tag="mm", name=f"tp_{j}_{pc}")
                for i in range(2):
                    for qs in range(NSB):
                        nc.tensor.transpose(
                            tp[_ts(i, 64), qs, :], O8s[pc, i][:, qs, :], id_sb,
                            tile_position=(0, 64 * i),
                        )
                nc.vector.tensor_copy(OT[:, pc, :, :], tp)

            emit_pv(0, 0)
            emit_norm(0, 0)
            emit_pv(0, 1)
            emit_norm(0, 1)
            emit_pv(1, 0)
            emit_norm(1, 0)
            emit_transposes(0)
            emit_pv(1, 1)
            emit_norm(1, 1)
            emit_transposes(1)
            if debug and j == 0:
                nc.sync.dma_start(out=dOT[:, :, :, :], in_=OT)

            for mc in range(8):
                dps = mmp.tile([128, 512], F32, tag="mm")
                for t in range(2):
                    nc.tensor.matmul(
                        dps[:, :],
                        lhsT=dn_sb[:, t, _ts(mc, 128)],
                        rhs=OT[:, t, :, :].rearrange("p a b -> p (a b)"),
                        start=(t == 0), stop=(t == 1),
                    )
                ev = evp.tile([128, 512], F16, tag="ev")
                # last block: alternate evac engine to shorten the drain tail
                if j == NSB - 1 and mc % 2 == 1:
                    nc.scalar.copy(ev, dps)
                else:
                    nc.vector.tensor_copy(ev, dps)
                nc.sync.dma_start(out=outT_r[:, mc, js], in_=ev)

        # startup: interleave weight-part and first-block x DMAs in
        # consumption order so the first projection matmuls start early
        xt0 = {}
        js0 = _ts(0, SBK)
        for xname, xsrc in (("q", xqT), ("k", xkT), ("v", xvT)):
            srcr = xsrc.rearrange("(c p) s -> p c s", p=128)
            wr = {"q": wqT, "k": wkT, "v": wvT}[xname].rearrange(
                "(c p) m -> p c m", p=128)
            tiles = []
            for kc2 in range(4):
                nc.sync.dma_start(out=wparts[xname][kc2], in_=wr[:, _ts(kc2, 2), :])
                t = xpool.tile([128, 2, SBK], F16, tag="xt",
                               name=f"x_{xname}_0_{kc2}")
                nc.sync.dma_start(out=t, in_=srcr[:, _ts(kc2, 2), js0])
                tiles.append(t)
            xt0[xname] = tiles
        nc.sync.dma_start(
            out=dn_sb, in_=dnT.rearrange("(t p) n -> p t n", p=128))
        nc.sync.dma_start(out=tri_sb, in_=tri_c[:, :])
        nc.sync.dma_start(out=id_sb, in_=id_c[:, :])

        xts = {0: xt0}
        for j in range(NSB):
            if j + 1 < NSB:
                xts[j + 1] = load_x(j + 1, _ts(j + 1, SBK))  # prefetch
            phase_A(j, _ts(j, SBK), xt=xts.pop(j))
        if debug:
            nc.sync.dma_start(out=dQT[:, :, :], in_=QT_sb)
            nc.sync.dma_start(out=dKT[:, :, :], in_=KT_sb)
            nc.sync.dma_start(out=dV[:, :, :, :], in_=V_sb)
        pts = {0: phase_LB(0, _ts(0, SBK))}
        for j in range(NSB):
            if j + 1 < NSB:
                pts[j + 1] = phase_LB(j + 1, _ts(j + 1, SBK))
            phase_PB(j, _ts(j, SBK), pts.pop(j))

    nc.finalize()
    return nc


_CACHE = {}


def _get_nc(causal=True, with_bq=False, with_bk=False, with_bv=False):
    key = (causal, with_bq, with_bk, with_bv)
    if key not in _CACHE:
        assert causal and not (with_bq or with_bk or with_bv)
        _CACHE[key] = build()
    return _CACHE[key]


def _numpy_fallback(query, key_, value, mask, wq_w, wq_b, wk_w, wk_b, wv_w,
                    wv_b, dense_w, dense_b):
    out = np.empty((B, S, D), np.float32)
    m4 = np.asarray(mask, np.float32).reshape(-1, S, S)
    for b in range(B):
        q = (query[b] @ wq_w.T + wq_b).reshape(S, H, DH).transpose(1, 0, 2)
        k = (key_[b] @ wk_w.T + wk_b).reshape(S, H, DH).transpose(1, 0, 2)
        v = (value[b] @ wv_w.T + wv_b).reshape(S, H, DH).transpose(1, 0, 2)
        mb = m4[min(b, m4.shape[0] - 1)]
        o = np.empty((H, S, DH), np.float32)
        for h in range(H):
            lg = (q[h] @ k[h].T) / np.sqrt(np.float32(DH)) + mb * np.float32(-1e9)
            lg -= lg.max(-1, keepdims=True)
            p = np.exp(lg)
            p /= p.sum(-1, keepdims=True)
            o[h] = p @ v[h]
        out[b] = o.transpose(1, 0, 2).reshape(S, D) @ dense_w.T + dense_b
    return out


def _prep_in_maps(query, key_, value, wq_w, wk_w, wv_w, dense_w):
    xT = {}
    for b in range(B):
        xT[b] = (
            np.ascontiguousarray(query[b].T).astype(np.float16),
            np.ascontiguousarray(key_[b].T).astype(np.float16),
            np.ascontiguousarray(value[b].T).astype(np.float16),
        )
    in_maps = []
    for c in range(NCORES):
        b, g = divmod(c, 4)
        sl = _ts(g, DLOC)
        in_maps.append({
            "xqT": xT[b][0], "xkT": xT[b][1], "xvT": xT[b][2],
            "wqT": np.ascontiguousarray(wq_w[sl].T).astype(np.float16),
            "wkT": np.ascontiguousarray(wk_w[sl].T).astype(np.float16),
            "wvT": np.ascontiguousarray(wv_w[sl].T).astype(np.float16),
            "dnT": np.ascontiguousarray(dense_w[:, sl].T).astype(np.float16),
        })
    return in_maps


def kernel(query, key_, value, mask, wq_w, wq_b, wk_w, wk_b, wv_w, wv_b,
           dense_w, dense_b, _profile_kw=None):
    query = np.asarray(query, np.float32)
    key_ = np.asarray(key_, np.float32)
    value = np.asarray(value, np.float32)
    mask2d = np.asarray(mask, np.float32).reshape(S, S)
    wq_w = np.asarray(wq_w, np.float32)
    wk_w = np.asarray(wk_w, np.float32)
    wv_w = np.asarray(wv_w, np.float32)
    dense_w = np.asarray(dense_w, np.float32)
    dense_b = np.asarray(dense_b, np.float32)

    causal = bool(np.array_equal(mask2d, np.triu(np.ones((S, S), np.float32), k=1)))
    if not causal or np.any(wq_b) or np.any(wk_b) or np.any(wv_b):
        out = _numpy_fallback(query, key_, value, mask, wq_w, wq_b, wk_w,
                              wk_b, wv_w, wv_b, dense_w, dense_b)
        return (out, None) if _profile_kw else out

    in_maps = _prep_in_maps(query, key_, value, wq_w, wk_w, wv_w, dense_w)
    nc = _get_nc(True, False, False, False)
    res = run_bass_kernel_spmd(nc, in_maps, core_ids=list(range(NCORES)),
                               **(_profile_kw or {}))

    out = np.empty((B, S, D), np.float32)
    for b in range(B):
        acc = res.results[4 * b]["outT"].astype(np.float32)
        for g in range(1, 4):
            acc = acc + res.results[4 * b + g]["outT"].astype(np.float32)
        out[b] = acc.T + dense_b[None, :]
    if _profile_kw:
        return out, res
    return out
